# revision 55
# baseline (speedup 1.0000x reference)
"""TRN2 Bass kernel for nn_CharModel (segment-mean over char ranges + pos embedding).

Strategy (pure data-parallel over batch, 8 cores x 4 batches):
  - Words are contiguous char ranges [start, start+L). Host sorts each batch's
    words by length L desc; same-length words across the core's 4 batches are
    packed (bl-major) into 128-partition columns. Per length L ONE
    indirect_dma_start call gathers every word's L rows as a single contiguous
    descriptor (L*768 bf16 elements via an overlapping row view) into
    [128, ncols_L, L*768] -- ~2.7k descriptors per core on the library-free
    INDIRECT1D q7 path, full-width so they spread evenly over all 16 SDMA
    engines.
  - feats is cast to bf16 on the host, halving HBM read traffic; sums
    accumulate in fp32 on DVE (L-1 adds per column), so only the per-element
    bf16 input rounding (~2^-9 relative) is lost.
  - SPMD runs one program on 8 cores: only the column COUNT per L is unified
    (max over cores); which word sits in which slot is per-core input data.
    Pad slots gather real rows (cheap, valid) and are neutralized by recip=0
    and a zero one-hot column; their output rows are discarded on the host.
  - Pos embedding via a host-built one-hot bf16 matmul on PE (PSUM), fused
    with the 1/len scaling in one scalar_tensor_tensor per column:
       out = (acc * recip) + psum_pos
  - Each column's finished [cu, 768] fp32 block DMAs straight to its row
    range of a flat output tensor; host scatters rows back to (batch, word)
    order. Rows the device never computes (len-0/invalid words) are exactly
    the pos-embedding row, filled on the host from the fp32 table.
"""

import numpy as np

B, S, W, D, PV = 32, 2048, 512, 768, 64
N_CORES = 8
BPC = B // N_CORES          # batches per core
P = 128
KMAX_DEVICE = 16            # device path supports word len up to this

LAST_RESULTS = None         # BassKernelResults of the most recent run (for test.py)


def _run_spmd(nc, in_maps, core_ids):
    """Indirection point so tests can swap in a simulator."""
    from concourse.bass_utils import run_bass_kernel_spmd
    return run_bass_kernel_spmd(nc, in_maps, core_ids)


def _word_ranges(word_lens, pos, seq_len):
    """Replicate the reference's starts/ends/valid computation in numpy."""
    wl = np.asarray(word_lens, np.int64)
    po = np.asarray(pos, np.int64)
    sl = np.asarray(seq_len, np.int64)
    b, w = wl.shape
    j = np.arange(w)
    next_start = np.concatenate([wl[:, 1:], np.zeros((b, 1), np.int64)], axis=1)
    is_last = (j[None, :] == w - 1) | (next_start == 0)
    starts = wl
    ends = np.where(is_last, sl[:, None], next_start)
    valid = (wl != 0) | (j[None, :] == 0)
    lens = np.where(valid, np.maximum(ends - starts, 0), 0)
    denom = np.maximum(ends - starts, 1).astype(np.float64)
    recip = np.where(valid & (lens > 0), 1.0 / denom, 0.0).astype(np.float32)
    return starts, lens, recip, po


def _numpy_fallback(feats, pos_table, word_lens, pos, seq_len):
    feats = np.asarray(feats, np.float32)
    pos_table = np.asarray(pos_table, np.float32)
    starts, lens, recip, po = _word_ranges(word_lens, pos, seq_len)
    out = np.zeros((feats.shape[0], po.shape[1], feats.shape[2]), np.float32)
    for b in range(out.shape[0]):
        for w in range(out.shape[1]):
            L = int(lens[b, w])
            if L > 0:
                s = int(starts[b, w])
                out[b, w] = feats[b, s:s + L].sum(axis=0) * recip[b, w]
        out[b] += pos_table[po[b]]
    return out


def _concourse_importable():
    try:
        import concourse.bass  # noqa: F401
        return True
    except ImportError:
        import sys
        for p in ("/opt/trn_rl_repo", "/root/.axon_site/_ro/trn_rl_repo"):
            if p not in sys.path:
                sys.path.append(p)
        try:
            import concourse.bass  # noqa: F401
            return True
        except ImportError:
            return False


def _prepare(feats, pos_table_np, starts, lens, recip, po, kmax):
    """Host-side layout.

    Returns (geom, in_maps, meta, tot_rows):
      geom: list of (L, colbase, ncols, cu_list) in descending-L order —
            the shared program shape.
      meta[core]: extraction records (bg, word_idx_array, out_row_start).
    """
    perms = np.zeros((B, W), np.int64)
    for b in range(B):
        perms[b] = np.argsort(-lens[b], kind="stable")
    # per (core, L): words bl-major in sorted order
    SL = np.zeros((N_CORES, kmax + 1), np.int64)
    for c in range(N_CORES):
        for L in range(1, kmax + 1):
            SL[c, L] = int((lens[c * BPC:(c + 1) * BPC] == L).sum())
    MS = SL.max(axis=0)                      # unified capacity per L

    geom = []
    colbase = 0
    rowbase = 0
    order = [1] + list(range(kmax, 1, -1))
    for L in order:
        if L > kmax or MS[L] == 0:
            continue
        ms = int(MS[L])
        ncols = -(-ms // P)
        cu_list = [min(P, ms - c * P) for c in range(ncols)]
        geom.append((L, colbase, ncols, cu_list, rowbase, ms))
        colbase += ncols
        rowbase += ms
    ncol_total = colbase
    tot_rows = rowbase

    # column processing order: pair low-L with high-L columns so the DVE
    # tree work per column stays level (avoids bursty add phases)
    cols_flat = []
    for (L, cb, ncols, cu_list, rb, ms) in geom:
        rowoff = rb
        for c in range(ncols):
            cols_flat.append((L, cb + c, c, cu_list[c], rowoff))
            rowoff += cu_list[c]
    by_l = sorted(cols_flat, key=lambda t: t[0])
    colorder = []
    lo, hi = 0, len(by_l) - 1
    while lo <= hi:
        colorder.append(by_l[lo])
        lo += 1
        if lo <= hi:
            colorder.append(by_l[hi])
            hi -= 1

    in_maps = []
    meta = []
    for core in range(N_CORES):
        bs = slice(core * BPC, (core + 1) * BPC)
        feats_h = feats[bs].reshape(-1, D).astype(np.float16)
        # int16 wrapped gather indices: per L a [128, 8*ncols] block where
        # element (p, c) = flat[c*16 + p%16] (16-wrapped, replicated x8 cores)
        idx32 = np.zeros((P, ncol_total), np.int32)
        recipv = np.zeros((P, ncol_total), np.float32)     # per-slot 1/len
        posid = np.full(ncol_total * P, -1.0, np.float16)  # per-slot pos id
        recs = []
        for (L, cb, ncols, cu_list, rb, ms) in geom:
            cap = P * ncols
            # pad slots read valid (but unused) rows spread over batch 0
            flat = np.zeros(cap, np.int64)
            flat[:] = (np.arange(cap) * 53) % (S - KMAX_DEVICE)
            slot = 0                         # slot index within this L block
            for bl in range(BPC):
                bg = core * BPC + bl
                perm = perms[bg]
                lsort = lens[bg][perm]
                gstart = int(np.searchsorted(-lsort, -L, side="left"))
                n_here = int((lens[bg] == L).sum())
                if n_here == 0:
                    continue
                wsel = perm[gstart:gstart + n_here]
                sl_idx = slot + np.arange(n_here)
                flat[sl_idx] = bl * S + starts[bg][wsel]
                pcol = sl_idx % P
                ccol = sl_idx // P
                recipv[pcol, cb + ccol] = recip[bg][wsel]
                posid[(cb + ccol) * P + pcol] = po[bg][wsel]
                recs.append((bg, wsel, rb + slot))
                slot += n_here
            idx32[:, cb:cb + ncols] = flat.reshape(ncols, P).T.astype(np.int32)
        in_maps.append({
            "feats_h": feats_h,
            "pos_tab": pos_table_np.astype(np.float16),
            "idx32": idx32,
            "recipv": recipv,
            "posid": np.broadcast_to(posid, (PV, ncol_total * P)).copy(),
            "diag1": np.eye(P, dtype=np.float16),
            "viota": np.arange(PV, dtype=np.float32).reshape(PV, 1),
        })
        meta.append(recs)
    return geom, colorder, ncol_total, in_maps, meta, tot_rows


def _build_nc(geom, colorder, ncol_total, tot_rows, kmax):
    from concourse import bass, bacc, mybir
    import concourse.tile as tile

    nrows = BPC * S
    nc = bacc.Bacc("TRN2", target_bir_lowering=False, debug=False)
    t_feats = nc.dram_tensor("feats_h", [nrows, D], mybir.dt.float16,
                             kind="ExternalInput")
    t_pos = nc.dram_tensor("pos_tab", [PV, D], mybir.dt.float16,
                           kind="ExternalInput")
    t_idx = nc.dram_tensor("idx32", [P, ncol_total], mybir.dt.int32,
                           kind="ExternalInput")
    t_recipv = nc.dram_tensor("recipv", [P, ncol_total], mybir.dt.float32,
                              kind="ExternalInput")
    t_posid = nc.dram_tensor("posid", [PV, ncol_total * P], mybir.dt.float16,
                             kind="ExternalInput")
    t_diag1 = nc.dram_tensor("diag1", [P, P], mybir.dt.float16,
                             kind="ExternalInput")
    t_viota = nc.dram_tensor("viota", [PV, 1], mybir.dt.float32,
                             kind="ExternalInput")
    t_out = nc.dram_tensor("out", [tot_rows, D], mybir.dt.float16,
                           kind="ExternalOutput")



    with tile.TileContext(nc) as tc:
        with (
            tc.tile_pool(name="const", bufs=1) as cpool,
            tc.tile_pool(name="gath", bufs=1) as gpool,
            tc.tile_pool(name="osb", bufs=6) as opool,
            tc.tile_pool(name="psum", bufs=4, space="PSUM") as ppool,
        ):
            idx_sb = cpool.tile([P, ncol_total], mybir.dt.int32)
            recipv_sb = cpool.tile([P, ncol_total], mybir.dt.float32)
            pos_sb = cpool.tile([PV, D], mybir.dt.float16)
            posid_sb = cpool.tile([PV, ncol_total * P], mybir.dt.float16)
            diag1_sb = cpool.tile([P, P], mybir.dt.float16)
            viota_sb = cpool.tile([PV, 1], mybir.dt.float32)
            oh_sb = cpool.tile([PV, ncol_total * P], mybir.dt.float16)
            recd_sb = cpool.tile([P, ncol_total * P], mybir.dt.float16)
            nc.sync.dma_start(out=idx_sb[:], in_=t_idx[:])
            nc.sync.dma_start(out=recipv_sb[:], in_=t_recipv[:])
            nc.sync.dma_start(out=diag1_sb[:], in_=t_diag1[:])
            nc.sync.dma_start(out=viota_sb[:], in_=t_viota[:])
            nc.sync.dma_start(out=pos_sb[:], in_=t_pos[:])
            nc.sync.dma_start(out=posid_sb[:], in_=t_posid[:])

            # All gathers on the library-free INDIRECT1D path: the in_ view is
            # the plain row-stride AP (correct q7 address coefficient D); the
            # descriptor LENGTH is destination-driven, so an L*D-wide dest row
            # pulls the word's L consecutive rows in one descriptor.
            gts = {}
            for (L, cb, ncols, cu_list, rb, ms) in geom:
                gt = gpool.tile([P, ncols, L * D], mybir.dt.float16,
                                tag=f"g{L}")
                gts[L] = gt
            for (L, k, c, cu, rowoff) in colorder:
                gt = gts[L]
                nc.gpsimd.indirect_dma_start(
                    out=gt[0:cu, c, :],
                    out_offset=None,
                    in_=t_feats[:],
                    in_offset=bass.IndirectOffsetOnAxis(
                        ap=idx_sb[0:cu, k:k + 1], axis=0),
                )

            # build the one-hot and diag(recip) tables on-device during the
            # gather ramp: oh[v, slot] = (posid[slot] == v), and per column
            # recd = diag1 * recip
            nc.vector.tensor_single_scalar(
                out=oh_sb[:, :], in_=posid_sb[:, :], scalar=viota_sb[:, 0:1],
                op=mybir.AluOpType.is_equal)
            for (L, k, c, cu, rowoff) in colorder:
                nc.vector.tensor_scalar_mul(
                    out=recd_sb[:, k * P:(k + 1) * P], in0=diag1_sb[:, :],
                    scalar1=recipv_sb[:, k:k + 1])

            colwork = colorder

            pending = []          # columns whose pos-matmul ran, diag pending
            done = []             # closed psums awaiting ACT drain

            def diag_close():
                L, k, c, cu, rowoff, psum = pending.pop(0)
                gt = gts[L]
                dg = recd_sb[0:cu, k * P:k * P + cu]
                nc.tensor.matmul(out=psum[0:cu, 0:512], lhsT=dg,
                                 rhs=gt[0:cu, c, 0:512], start=False,
                                 stop=True)
                nc.tensor.matmul(out=psum[0:cu, 512:D], lhsT=dg,
                                 rhs=gt[0:cu, c, 512:D], start=False,
                                 stop=True)
                done.append((psum, cu, rowoff))

            def drain_one():
                psum, cu, rowoff = done.pop(0)
                osb = opool.tile([P, D], mybir.dt.float16, tag="osb")
                nc.scalar.activation(out=osb[0:cu, :], in_=psum[0:cu, :],
                                     func=mybir.ActivationFunctionType.Copy)
                nc.sync.dma_start(out=t_out[rowoff:rowoff + cu, :],
                                  in_=osb[0:cu, :])

            for (L, k, c, cu, rowoff) in colwork:
                gt = gts[L]

                def row(r):
                    return gt[0:cu, c, r * D:(r + 1) * D]

                step = 1           # pairwise in-place fold: result in row 0
                while step < L:
                    for i in range(0, L - step, 2 * step):
                        nc.vector.tensor_add(out=row(i), in0=row(i),
                                             in1=row(i + step))
                    step *= 2
                psum = ppool.tile([P, D], mybir.dt.float32, space="PSUM",
                                  tag="ps")
                lhs = oh_sb[:, k * P:k * P + cu]
                nc.tensor.matmul(out=psum[0:cu, 0:512], lhsT=lhs,
                                 rhs=pos_sb[:, 0:512], start=True, stop=False)
                nc.tensor.matmul(out=psum[0:cu, 512:D], lhsT=lhs,
                                 rhs=pos_sb[:, 512:D], start=True, stop=False)
                pending.append((L, k, c, cu, rowoff, psum))
                if len(pending) > 1:
                    diag_close()
                while done:
                    drain_one()
            while pending:
                diag_close()
            while done:
                drain_one()
    nc.finalize()
    return nc


def kernel(feats, pos_table, word_lens, pos, seq_len):
    global LAST_RESULTS
    feats = np.ascontiguousarray(np.asarray(feats, np.float32))
    pos_table_np = np.ascontiguousarray(np.asarray(pos_table, np.float32))
    starts, lens, recip, po = _word_ranges(word_lens, pos, seq_len)

    kmax = int(lens.max())
    shapes_ok = (
        feats.shape == (B, S, D)
        and pos_table_np.shape == (PV, D)
        and po.shape == (B, W)
        and starts.shape == (B, W)
        and np.asarray(seq_len).shape == (B,)
        and int(po.max()) < PV and int(po.min()) >= 0
    )
    if kmax > KMAX_DEVICE or kmax < 1 or not shapes_ok \
            or not _concourse_importable():
        return _numpy_fallback(feats, pos_table, word_lens, pos, seq_len)

    geom, colorder, ncol_total, in_maps, meta, tot_rows = _prepare(
        feats, pos_table_np, starts, lens, recip, po, kmax)
    nc = _build_nc(geom, colorder, ncol_total, tot_rows, kmax)

    res = _run_spmd(nc, in_maps, list(range(N_CORES)))
    LAST_RESULTS = res

    out = np.zeros((B, W, D), np.float32)
    for core in range(N_CORES):
        arr = res.results[core]["out"]            # [tot_rows, D]
        for bg, wsel, rowstart in meta[core]:
            out[bg][wsel] = arr[rowstart:rowstart + len(wsel)]
    # slots the device never computes: invalid words and len-0 words get
    # means == 0, so the exact answer is just the pos embedding row
    zmask = lens == 0
    if zmask.any():
        out[zmask] = pos_table_np[po[zmask]]
    return out


# revision 56
# speedup vs baseline: 1.0366x; 1.0366x over previous
"""TRN2 Bass kernel for nn_CharModel (segment-mean over char ranges + pos embedding).

Strategy (pure data-parallel over batch, 8 cores x 4 batches):
  - Words are contiguous char ranges [start, start+L). Host sorts each batch's
    words by length L desc; same-length words across the core's 4 batches are
    packed (bl-major) into 128-partition columns. Per length L ONE
    indirect_dma_start call gathers every word's L rows as a single contiguous
    descriptor (L*768 bf16 elements via an overlapping row view) into
    [128, ncols_L, L*768] -- ~2.7k descriptors per core on the library-free
    INDIRECT1D q7 path, full-width so they spread evenly over all 16 SDMA
    engines.
  - feats is cast to bf16 on the host, halving HBM read traffic; sums
    accumulate in fp32 on DVE (L-1 adds per column), so only the per-element
    bf16 input rounding (~2^-9 relative) is lost.
  - SPMD runs one program on 8 cores: only the column COUNT per L is unified
    (max over cores); which word sits in which slot is per-core input data.
    Pad slots gather real rows (cheap, valid) and are neutralized by recip=0
    and a zero one-hot column; their output rows are discarded on the host.
  - Pos embedding via a host-built one-hot bf16 matmul on PE (PSUM), fused
    with the 1/len scaling in one scalar_tensor_tensor per column:
       out = (acc * recip) + psum_pos
  - Each column's finished [cu, 768] fp32 block DMAs straight to its row
    range of a flat output tensor; host scatters rows back to (batch, word)
    order. Rows the device never computes (len-0/invalid words) are exactly
    the pos-embedding row, filled on the host from the fp32 table.
"""

import numpy as np

B, S, W, D, PV = 32, 2048, 512, 768, 64
N_CORES = 8
BPC = B // N_CORES          # batches per core
P = 128
KMAX_DEVICE = 16            # device path supports word len up to this

LAST_RESULTS = None         # BassKernelResults of the most recent run (for test.py)


def _run_spmd(nc, in_maps, core_ids):
    """Indirection point so tests can swap in a simulator."""
    from concourse.bass_utils import run_bass_kernel_spmd
    return run_bass_kernel_spmd(nc, in_maps, core_ids)


def _word_ranges(word_lens, pos, seq_len):
    """Replicate the reference's starts/ends/valid computation in numpy."""
    wl = np.asarray(word_lens, np.int64)
    po = np.asarray(pos, np.int64)
    sl = np.asarray(seq_len, np.int64)
    b, w = wl.shape
    j = np.arange(w)
    next_start = np.concatenate([wl[:, 1:], np.zeros((b, 1), np.int64)], axis=1)
    is_last = (j[None, :] == w - 1) | (next_start == 0)
    starts = wl
    ends = np.where(is_last, sl[:, None], next_start)
    valid = (wl != 0) | (j[None, :] == 0)
    lens = np.where(valid, np.maximum(ends - starts, 0), 0)
    denom = np.maximum(ends - starts, 1).astype(np.float64)
    recip = np.where(valid & (lens > 0), 1.0 / denom, 0.0).astype(np.float32)
    return starts, lens, recip, po


def _numpy_fallback(feats, pos_table, word_lens, pos, seq_len):
    feats = np.asarray(feats, np.float32)
    pos_table = np.asarray(pos_table, np.float32)
    starts, lens, recip, po = _word_ranges(word_lens, pos, seq_len)
    out = np.zeros((feats.shape[0], po.shape[1], feats.shape[2]), np.float32)
    for b in range(out.shape[0]):
        for w in range(out.shape[1]):
            L = int(lens[b, w])
            if L > 0:
                s = int(starts[b, w])
                out[b, w] = feats[b, s:s + L].sum(axis=0) * recip[b, w]
        out[b] += pos_table[po[b]]
    return out


def _concourse_importable():
    try:
        import concourse.bass  # noqa: F401
        return True
    except ImportError:
        import sys
        for p in ("/opt/trn_rl_repo", "/root/.axon_site/_ro/trn_rl_repo"):
            if p not in sys.path:
                sys.path.append(p)
        try:
            import concourse.bass  # noqa: F401
            return True
        except ImportError:
            return False


def _prepare(feats, pos_table_np, starts, lens, recip, po, kmax):
    """Host-side layout.

    Returns (geom, in_maps, meta, tot_rows):
      geom: list of (L, colbase, ncols, cu_list) in descending-L order —
            the shared program shape.
      meta[core]: extraction records (bg, word_idx_array, out_row_start).
    """
    perms = np.zeros((B, W), np.int64)
    for b in range(B):
        perms[b] = np.argsort(-lens[b], kind="stable")
    # per (core, L): words bl-major in sorted order
    SL = np.zeros((N_CORES, kmax + 1), np.int64)
    for c in range(N_CORES):
        for L in range(1, kmax + 1):
            SL[c, L] = int((lens[c * BPC:(c + 1) * BPC] == L).sum())
    MS = SL.max(axis=0)                      # unified capacity per L

    geom = []
    colbase = 0
    rowbase = 0
    order = [1] + list(range(kmax, 1, -1))
    for L in order:
        if L > kmax or MS[L] == 0:
            continue
        ms = int(MS[L])
        ncols = -(-ms // P)
        cu_list = [min(P, ms - c * P) for c in range(ncols)]
        geom.append((L, colbase, ncols, cu_list, rowbase, ms))
        colbase += ncols
        rowbase += ms
    ncol_total = colbase
    tot_rows = rowbase

    # column processing order: pair low-L with high-L columns so the DVE
    # tree work per column stays level (avoids bursty add phases)
    cols_flat = []
    for (L, cb, ncols, cu_list, rb, ms) in geom:
        rowoff = rb
        for c in range(ncols):
            cols_flat.append((L, cb + c, c, cu_list[c], rowoff))
            rowoff += cu_list[c]
    by_l = sorted(cols_flat, key=lambda t: t[0])
    colorder = []
    lo, hi = 0, len(by_l) - 1
    while lo <= hi:
        colorder.append(by_l[lo])
        lo += 1
        if lo <= hi:
            colorder.append(by_l[hi])
            hi -= 1

    in_maps = []
    meta = []
    for core in range(N_CORES):
        bs = slice(core * BPC, (core + 1) * BPC)
        feats_h = feats[bs].reshape(-1, D).astype(np.float16)
        # int16 wrapped gather indices: per L a [128, 8*ncols] block where
        # element (p, c) = flat[c*16 + p%16] (16-wrapped, replicated x8 cores)
        idx32 = np.zeros((P, ncol_total), np.int32)
        recipv = np.zeros((P, ncol_total), np.float32)     # per-slot 1/len
        posid = np.full(ncol_total * P, -1.0, np.float16)  # per-slot pos id
        recs = []
        for (L, cb, ncols, cu_list, rb, ms) in geom:
            cap = P * ncols
            # pad slots read valid (but unused) rows spread over batch 0
            flat = np.zeros(cap, np.int64)
            flat[:] = (np.arange(cap) * 53) % (S - KMAX_DEVICE)
            slot = 0                         # slot index within this L block
            for bl in range(BPC):
                bg = core * BPC + bl
                perm = perms[bg]
                lsort = lens[bg][perm]
                gstart = int(np.searchsorted(-lsort, -L, side="left"))
                n_here = int((lens[bg] == L).sum())
                if n_here == 0:
                    continue
                wsel = perm[gstart:gstart + n_here]
                sl_idx = slot + np.arange(n_here)
                flat[sl_idx] = bl * S + starts[bg][wsel]
                pcol = sl_idx % P
                ccol = sl_idx // P
                recipv[pcol, cb + ccol] = recip[bg][wsel]
                posid[(cb + ccol) * P + pcol] = po[bg][wsel]
                recs.append((bg, wsel, rb + slot))
                slot += n_here
            idx32[:, cb:cb + ncols] = flat.reshape(ncols, P).T.astype(np.int32)
        in_maps.append({
            "feats_h": feats_h,
            "pos_tab": pos_table_np.astype(np.float16),
            "idx32": idx32,
            "recipv": recipv,
            "posid": np.broadcast_to(posid, (PV, ncol_total * P)).copy(),
            "diag1": np.eye(P, dtype=np.float16),
            "viota": np.arange(PV, dtype=np.float32).reshape(PV, 1),
        })
        meta.append(recs)
    return geom, colorder, ncol_total, in_maps, meta, tot_rows


def _build_nc(geom, colorder, ncol_total, tot_rows, kmax):
    from concourse import bass, bacc, mybir
    import concourse.tile as tile

    nrows = BPC * S
    nc = bacc.Bacc("TRN2", target_bir_lowering=False, debug=False)
    t_feats = nc.dram_tensor("feats_h", [nrows, D], mybir.dt.float16,
                             kind="ExternalInput")
    t_pos = nc.dram_tensor("pos_tab", [PV, D], mybir.dt.float16,
                           kind="ExternalInput")
    t_idx = nc.dram_tensor("idx32", [P, ncol_total], mybir.dt.int32,
                           kind="ExternalInput")
    t_recipv = nc.dram_tensor("recipv", [P, ncol_total], mybir.dt.float32,
                              kind="ExternalInput")
    t_posid = nc.dram_tensor("posid", [PV, ncol_total * P], mybir.dt.float16,
                             kind="ExternalInput")
    t_diag1 = nc.dram_tensor("diag1", [P, P], mybir.dt.float16,
                             kind="ExternalInput")
    t_viota = nc.dram_tensor("viota", [PV, 1], mybir.dt.float32,
                             kind="ExternalInput")
    t_out = nc.dram_tensor("out", [tot_rows, D], mybir.dt.float16,
                           kind="ExternalOutput")



    with tile.TileContext(nc) as tc:
        with (
            tc.tile_pool(name="const", bufs=1) as cpool,
            tc.tile_pool(name="gath", bufs=1) as gpool,
            tc.tile_pool(name="osb", bufs=6) as opool,
            tc.tile_pool(name="psum", bufs=4, space="PSUM") as ppool,
        ):
            idx_sb = cpool.tile([P, ncol_total], mybir.dt.int32)
            recipv_sb = cpool.tile([P, ncol_total], mybir.dt.float32)
            pos_sb = cpool.tile([PV, D], mybir.dt.float16)
            posid_sb = cpool.tile([PV, ncol_total * P], mybir.dt.float16)
            diag1_sb = cpool.tile([P, P], mybir.dt.float16)
            viota_sb = cpool.tile([PV, 1], mybir.dt.float32)
            oh_sb = cpool.tile([PV, ncol_total * P], mybir.dt.float16)
            recd_sb = cpool.tile([P, ncol_total * P], mybir.dt.float16)
            nc.sync.dma_start(out=idx_sb[:], in_=t_idx[:])
            nc.sync.dma_start(out=recipv_sb[:], in_=t_recipv[:])
            nc.sync.dma_start(out=diag1_sb[:], in_=t_diag1[:])
            nc.sync.dma_start(out=viota_sb[:], in_=t_viota[:])
            nc.sync.dma_start(out=pos_sb[:], in_=t_pos[:])
            nc.sync.dma_start(out=posid_sb[:], in_=t_posid[:])

            # All gathers on the library-free INDIRECT1D path: the in_ view is
            # the plain row-stride AP (correct q7 address coefficient D); the
            # descriptor LENGTH is destination-driven, so an L*D-wide dest row
            # pulls the word's L consecutive rows in one descriptor.
            gts = {}
            for (L, cb, ncols, cu_list, rb, ms) in geom:
                gt = gpool.tile([P, ncols, L * D], mybir.dt.float16,
                                tag=f"g{L}")
                gts[L] = gt
            for (L, k, c, cu, rowoff) in colorder:
                gt = gts[L]
                nc.gpsimd.indirect_dma_start(
                    out=gt[0:cu, c, :],
                    out_offset=None,
                    in_=t_feats[:],
                    in_offset=bass.IndirectOffsetOnAxis(
                        ap=idx_sb[0:cu, k:k + 1], axis=0),
                )

            # build the one-hot and diag(recip) tables on-device during the
            # gather ramp: oh[v, slot] = (posid[slot] == v), and per column
            # recd = diag1 * recip
            nc.vector.tensor_single_scalar(
                out=oh_sb[:, :], in_=posid_sb[:, :], scalar=viota_sb[:, 0:1],
                op=mybir.AluOpType.is_equal)
            for (L, k, c, cu, rowoff) in colorder:
                nc.vector.tensor_scalar_mul(
                    out=recd_sb[:, k * P:(k + 1) * P], in0=diag1_sb[:, :],
                    scalar1=recipv_sb[:, k:k + 1])

            colwork = colorder

            pending = []          # columns whose pos-matmul ran, diag pending
            done = []             # closed psums awaiting ACT drain

            def diag_close():
                L, k, c, cu, rowoff, psum = pending.pop(0)
                gt = gts[L]
                dg = recd_sb[0:cu, k * P:k * P + cu]
                nc.tensor.matmul(out=psum[0:cu, 0:512], lhsT=dg,
                                 rhs=gt[0:cu, c, 0:512], start=False,
                                 stop=True)
                nc.tensor.matmul(out=psum[0:cu, 512:D], lhsT=dg,
                                 rhs=gt[0:cu, c, 512:D], start=False,
                                 stop=True)
                done.append((psum, cu, rowoff))

            def drain_one():
                psum, cu, rowoff = done.pop(0)
                osb = opool.tile([P, D], mybir.dt.float16, tag="osb")
                nc.scalar.activation(out=osb[0:cu, :], in_=psum[0:cu, :],
                                     func=mybir.ActivationFunctionType.Copy)
                nc.sync.dma_start(out=t_out[rowoff:rowoff + cu, :],
                                  in_=osb[0:cu, :])

            for (L, k, c, cu, rowoff) in colwork:
                gt = gts[L]

                def row(r):
                    return gt[0:cu, c, r * D:(r + 1) * D]

                step = 1           # pairwise in-place fold: result in row 0
                while step < L:
                    for i in range(0, L - step, 2 * step):
                        nc.vector.tensor_add(out=row(i), in0=row(i),
                                             in1=row(i + step))
                    step *= 2
                psum = ppool.tile([P, D], mybir.dt.float32, space="PSUM",
                                  tag="ps")
                lhs = oh_sb[:, k * P:k * P + cu]
                nc.tensor.matmul(out=psum[0:cu, 0:512], lhsT=lhs,
                                 rhs=pos_sb[:, 0:512], start=True, stop=False)
                nc.tensor.matmul(out=psum[0:cu, 512:D], lhsT=lhs,
                                 rhs=pos_sb[:, 512:D], start=True, stop=False)
                pending.append((L, k, c, cu, rowoff, psum))
                if len(pending) > 1:
                    diag_close()
                if len(done) > 1:
                    drain_one()
            while pending:
                diag_close()
            while done:
                drain_one()
    nc.finalize()
    return nc


def kernel(feats, pos_table, word_lens, pos, seq_len):
    global LAST_RESULTS
    feats = np.ascontiguousarray(np.asarray(feats, np.float32))
    pos_table_np = np.ascontiguousarray(np.asarray(pos_table, np.float32))
    starts, lens, recip, po = _word_ranges(word_lens, pos, seq_len)

    kmax = int(lens.max())
    shapes_ok = (
        feats.shape == (B, S, D)
        and pos_table_np.shape == (PV, D)
        and po.shape == (B, W)
        and starts.shape == (B, W)
        and np.asarray(seq_len).shape == (B,)
        and int(po.max()) < PV and int(po.min()) >= 0
    )
    if kmax > KMAX_DEVICE or kmax < 1 or not shapes_ok \
            or not _concourse_importable():
        return _numpy_fallback(feats, pos_table, word_lens, pos, seq_len)

    geom, colorder, ncol_total, in_maps, meta, tot_rows = _prepare(
        feats, pos_table_np, starts, lens, recip, po, kmax)
    nc = _build_nc(geom, colorder, ncol_total, tot_rows, kmax)

    res = _run_spmd(nc, in_maps, list(range(N_CORES)))
    LAST_RESULTS = res

    out = np.zeros((B, W, D), np.float32)
    for core in range(N_CORES):
        arr = res.results[core]["out"]            # [tot_rows, D]
        for bg, wsel, rowstart in meta[core]:
            out[bg][wsel] = arr[rowstart:rowstart + len(wsel)]
    # slots the device never computes: invalid words and len-0 words get
    # means == 0, so the exact answer is just the pos embedding row
    zmask = lens == 0
    if zmask.any():
        out[zmask] = pos_table_np[po[zmask]]
    return out
